# revision 18
# baseline (speedup 1.0000x reference)
"""GCNModelAE social GNN kernel for 8 trn2 NeuronCores.

Sharding: head-sharded GAT (core h owns attention head h for ALL nodes,
zT = [key, query] score layout), column-sharded GCN/recon (core j owns
node columns j*256:(j+1)*256). AllGathers stitch per-head outputs into
the concat/mean feature maps between layers.
"""
import os
import sys
sys.path.insert(0, '/opt/trn_rl_repo')
import numpy as np
import ml_dtypes

import concourse.bass as bass
import concourse.bacc as bacc
import concourse.mybir as mybir
import concourse.tile as tile
from concourse.masks import make_identity
from concourse.bass_utils import run_bass_kernel_spmd

dt = mybir.dt
f32 = dt.float32
fp8 = dt.float8e4
Alu = mybir.AluOpType
Act = mybir.ActivationFunctionType

N = 2048
NKT = 16          # node k-tiles
NC = 8            # cores == heads
HD = 64           # hidden per head
M = 512
RC = N // NC      # 256 node columns per core
F1P, F2P = 2048, 1920   # padded feature dims
FK1, FK2 = F1P // 128, F2P // 128
ACT_LRELU_KT = 8  # how many of the 16 key-tiles run lrelu on ACT (rest DVE)

_cache = {}


# ---------------------------------------------------------------- host packing
def _t128(a):
    """[F, X] -> [128, F//128, X] (partition-major k-tiles)."""
    F, X = a.shape
    assert F % 128 == 0
    return np.ascontiguousarray(a.reshape(F // 128, 128, X).transpose(1, 0, 2))


def _pad_rows(a, F):
    out = np.zeros((F, a.shape[1]), np.float32)
    out[: a.shape[0]] = a
    return out


# ---------------------------------------------------------------- device build
def _build(debug=False):
    nc = bacc.Bacc("TRN2", target_bir_lowering=False, debug=False, num_devices=NC)

    # ---- inputs ----
    ins = {}

    def dram_in(name, shape, dtype=f32):
        ins[name] = nc.dram_tensor(name, shape, dtype, kind="ExternalInput")
        return ins[name]

    featT = {1: dram_in("featT1", [F1P, N]), 2: dram_in("featT2", [F2P, N])}
    mask8 = {1: dram_in("mask81", [128, NKT, N], fp8),
             2: dram_in("mask82", [128, NKT, N], fp8)}
    adjTj = {1: dram_in("adjTj1", [128, NKT, RC]), 2: dram_in("adjTj2", [128, NKT, RC])}
    sel = dram_in("sel", [128, NKT, RC])
    OH = {1: dram_in("OH1", [N, M]), 2: dram_in("OH2", [N, M])}
    FKg = {1: FK1, 2: FK2}

    WcatA, WcatB, Wgc = {}, {}, {}
    avec = {}
    for g in (1, 2):
        for r in (1, 2, 3):
            fka = FKg[g] if r == 1 else 1
            WcatA[g, r] = dram_in(f"WcatA{g}{r}", [128, fka, 128])
            WcatB[g, r] = dram_in(f"WcatB{g}{r}", [128, 4, 128])
            fkg = FKg[g] if r == 1 else 1
            Wgc[g, r] = dram_in(f"Wgc{g}{r}", [128, fkg, 64])
            for ab in "AB":
                for nm in ("a1", "a2", "b"):
                    key = f"{nm}{ab}{g}{r}"
                    avec[key] = dram_in(key, [64, 1])
    Wc = {g: dram_in(f"Wc{g}", [128, 64]) for g in (1, 2)}
    bc = {g: dram_in(f"bc{g}", [64, 1]) for g in (1, 2)}
    Wd1 = dram_in("Wd1", [128, 64]); bd1 = dram_in("bd1", [64, 1])
    Wlm = dram_in("Wlm", [64, 64]); blm = dram_in("blm", [64, 1])
    Wf = dram_in("Wf", [128, 128]); bf = dram_in("bf", [128, 1])
    Wo = dram_in("Wo", [128, 2]); bo = dram_in("bo", [2, 1])
    sel16_in = dram_in("sel16", [16, NKT * 64])

    # ---- outputs ----
    recon_out = {g: nc.dram_tensor(f"recon{g}", [N, RC], f32, kind="ExternalOutput")
                 for g in (1, 2)}
    head_out = nc.dram_tensor("headT", [2, M], f32, kind="ExternalOutput")
    dbg = {}

    def dbg_out(name, shape):
        if debug and name not in dbg:
            dbg[name] = nc.dram_tensor(f"dbg_{name}", shape, f32, kind="ExternalOutput")
        return dbg.get(name)

    with tile.TileContext(nc) as tc:
        from contextlib import ExitStack
        with ExitStack() as ctx:
            P = {}
            for nm, bufs in [("rhs", 2), ("fr", 2), ("f1b", 1), ("zml", 2),
                             ("v", 2), ("p", 2), ("lout", 3), ("aux", 2),
                             ("nat", 2), ("wts", 1), ("mask", 1), ("sm", 14),
                             ("me", 2), ("const", 1)]:
                P[nm] = ctx.enter_context(tc.tile_pool(name=nm, bufs=bufs))
            pmed = ctx.enter_context(tc.tile_pool(name="pmed", bufs=3, space="PSUM"))
            psm = ctx.enter_context(tc.tile_pool(name="psm", bufs=2, space="PSUM"))
            dram = ctx.enter_context(tc.tile_pool(name="dram", bufs=2, space="DRAM"))

            # ---- constants ----
            ident = P["const"].tile([128, 128], f32, name="ident", tag="ident")
            make_identity(nc, ident[:])
            ones64 = P["const"].tile([1, 64], f32, name="ones64", tag="ones64")
            nc.vector.memset(ones64[:], 1.0)
            # sel16[k, kt*64+o] = 1 if k == kt else 0 — row-selector weights
            sel16 = P["const"].tile([16, NKT * 64], f32, name="sel16", tag="sel16")
            nc.sync.dma_start(sel16[:], sel16_in[:])

            uid = [0]

            def nm(s):
                uid[0] += 1
                return f"{s}_{uid[0]}"

            def load_sm(src, shape, tag="sm"):
                t = P["sm"].tile(shape, f32, name=nm("sm"), tag=tag)
                nc.sync.dma_start(t[:], src[:])
                return t

            def load_sm64(src):
                """Load a [64,1] vector into partitions 64..127 of a tile."""
                t = P["sm"].tile([128, 1], f32, name=nm("sm64"), tag="sm")
                nc.sync.dma_start(t[64:128, :], src[:])
                return t

            # ---------------- helpers ----------------
            def mm_accum_to_sb(pfx, FK, get_rhs, lhsT_of, Mout, n_reads=1):
                """out_sb[Mout, N] = sum_kt lhsT(kt).T @ rhs(kt); chunked psum."""
                parts = [pmed.tile([128, 1024], f32, name=nm(f"{pfx}_ps{h}"),
                                   tag="pmed") for h in (0, 1)]
                for kt in range(FK):
                    rhs = get_rhs(kt)
                    for half in (0, 1):
                        for c in (0, 1):
                            sl = half * 1024 + c * 512
                            nc.tensor.matmul(
                                parts[half][:Mout, c * 512:(c + 1) * 512],
                                lhsT_of(kt), rhs[:, sl:sl + 512],
                                start=(kt == 0), stop=(kt == FK - 1),
                                skip_group_check=True)
                out_sb = P[pfx_pool[pfx]].tile(
                    [Mout if Mout > 65 else 65, N], f32,
                    name=nm(f"{pfx}_sb"), tag=pfx_pool_tag[pfx])
                for half in (0, 1):
                    nc.vector.tensor_copy(
                        out_sb[:Mout, half * 1024:(half + 1) * 1024],
                        parts[half][:Mout, :])
                return out_sb

            pfx_pool = {"fr": "fr", "xw": "aux", "f1b": "f1b", "cc": "aux",
                        "d1": "aux", "lat": "aux"}
            pfx_pool_tag = {"fr": "fr", "xw": "aux", "f1b": "f1b", "cc": "aux",
                            "d1": "aux", "lat": "aux"}

            def transposes(src_sb, with_ones):
                """src_sb [64, N] -> nat [128, NKT, 65|64] (+ones col)."""
                step = 65 if with_ones else 64
                nat = P["nat"].tile([128, NKT, step], f32, name=nm("nat"), tag="nat")
                if with_ones:
                    nc.vector.memset(nat[:, :, 64:65], 1.0)
                for kt in range(NKT):
                    tp = psm.tile([128, 64], f32, name=nm("tp"), tag="psm")
                    nc.tensor.transpose(tp[:], src_sb[:64, kt * 128:(kt + 1) * 128],
                                        ident[:64, :64])
                    nc.vector.tensor_copy(nat[:, kt, 0:64], tp[:])
                return nat

            def stream_rhs(dram_t, kt, rows=128, width=N):
                t = P["rhs"].tile([128, N], f32, name=nm("rhs"), tag="rhs")
                nc.sync.dma_start(t[:rows, :width],
                                  dram_t[kt * rows:(kt + 1) * rows, :])
                return t

            # ---------------- attention layer ----------------
            def attention(pfx, g, fr_sb, a1c, a2c, bcol, mask_sb, relu, is_b,
                          agin):
                ftsT = fr_sb[0:64, :]
                resT = fr_sb[64:128, :]
                # a1 replicated columns -> [64, 128]
                a1rep = P["sm"].tile([64, 128], f32, name=nm("a1rep"), tag="sm")
                nc.vector.tensor_copy(a1rep[:], a1c[:].broadcast_to([64, 128]))
                # f1b [128, N] (f1 per query broadcast over key partitions)
                f1b = mm_accum_to_sb("f1b", 1, lambda kt: ftsT,
                                     lambda kt: a1rep[:], 128)
                # f2T [128, NKT]
                f2p = psm.tile([128, NKT], f32, name=nm("f2p"), tag="psm")
                for kt in range(NKT):
                    nc.tensor.matmul(f2p[:, kt:kt + 1],
                                     ftsT[:, kt * 128:(kt + 1) * 128], a2c[:],
                                     start=True, stop=True, skip_group_check=True)
                f2s = P["sm"].tile([128, NKT], f32, name=nm("f2s"), tag="sm")
                nc.vector.tensor_copy(f2s[:], f2p[:])
                # fts nat + ones col
                ftso = transposes(ftsT, True)
                # attention main loop
                pv = [pmed.tile([65, 1024], f32, name=nm(f"{pfx}_pv{h}"),
                                tag="pmed") for h in (0, 1)]
                for kt in range(NKT):
                    zm = P["zml"].tile([128, N], f32, name=nm("zm"), tag="zml")
                    nc.vector.scalar_tensor_tensor(
                        zm[:], f1b[:128, :], f2s[:, kt:kt + 1], mask_sb[:, kt, :],
                        op0=Alu.add, op1=Alu.add)
                    l = P["zml"].tile([128, N], f32, name=nm("l"), tag="zml")
                    if kt < ACT_LRELU_KT:
                        nc.scalar.activation(l[:], zm[:], Act.Prelu,
                                             bias=0.0, scale=1.0, alpha=0.2)
                    else:
                        nc.vector.scalar_tensor_tensor(
                            l[:], zm[:], 0.2, zm[:], op0=Alu.mult, op1=Alu.max)
                    p = P["p"].tile([128, N], f32, name=nm("p"), tag="p")
                    nc.scalar.activation(p[:], l[:], Act.Exp)
                    for half in (0, 1):
                        for c in (0, 1):
                            sl = half * 1024 + c * 512
                            nc.tensor.matmul(
                                pv[half][:, c * 512:(c + 1) * 512],
                                ftso[:, kt, :], p[:, sl:sl + 512],
                                start=(kt == 0), stop=(kt == NKT - 1),
                                skip_group_check=True)
                pv_sb = P["aux"].tile([65, N], f32, name=nm(f"{pfx}_pvsb"),
                                      tag="aux")
                for half in (0, 1):
                    nc.vector.tensor_copy(pv_sb[:, half * 1024:(half + 1) * 1024],
                                          pv[half][:, :])
                # softmax denominators -> rsinv in row form [16, 128]
                s16 = psm.tile([128, NKT], f32, name=nm("s16"), tag="psm")
                for kt in range(NKT):
                    nc.tensor.transpose(s16[:, kt:kt + 1],
                                        pv_sb[64:65, kt * 128:(kt + 1) * 128],
                                        ident[64:65, 64:65])
                rsT = P["sm"].tile([128, NKT], f32, name=nm("rsT"), tag="sm")
                if is_b:  # fold the head-mean 1/8 into the reciprocal
                    s16s = P["sm"].tile([128, NKT], f32, name=nm("s16s"), tag="sm")
                    nc.vector.tensor_scalar(s16s[:], s16[:], 8.0, None, Alu.mult)
                    nc.vector.reciprocal(rsT[:], s16s[:])
                else:
                    nc.vector.reciprocal(rsT[:], s16[:])
                r16p = psm.tile([16, 128], f32, name=nm("r16p"), tag="psm")
                nc.tensor.transpose(r16p[:], rsT[:], ident[:, :])
                r16 = P["sm"].tile([16, 128], f32, name=nm("r16"), tag="sm")
                nc.vector.tensor_copy(r16[:], r16p[:])
                # epilogue per half: v = act((pv*rsinv) + b + res)
                v = P["v"].tile([128, N], f32, name=nm(f"{pfx}_v"), tag="v")
                for half in (0, 1):
                    rsb = pmed.tile([64, 1024], f32, name=nm("rsb"), tag="pmed")
                    for k8 in range(8):
                        kt = half * 8 + k8
                        nc.tensor.matmul(rsb[:, k8 * 128:(k8 + 1) * 128],
                                         sel16[:, kt * 64:(kt + 1) * 64],
                                         r16[:, :],
                                         start=True, stop=True,
                                         skip_group_check=True)
                    hs = slice(half * 1024, (half + 1) * 1024)
                    # all tensor-pairs share a base partition (HW constraint):
                    # v[64:128] = pv * rsinv ; v[0:64] = (that + b) + res
                    nc.vector.scalar_tensor_tensor(
                        v[64:128, hs], pv_sb[0:64, hs], 0.0, rsb[:],
                        op0=Alu.add, op1=Alu.mult)
                    nc.vector.scalar_tensor_tensor(
                        v[0:64, hs], v[64:128, hs], bcol[64:128, :],
                        resT[:, hs], op0=Alu.add, op1=Alu.add)
                    if relu:
                        nc.scalar.activation(v[64:128, hs], v[0:64, hs], Act.Relu)
                        nc.sync.dma_start(agin[0:64, hs], v[64:128, hs])
                    else:
                        nc.sync.dma_start(agin[0:64, hs], v[0:64, hs])
                return pv_sb, f1b, f2s

            def allgather(agin, rows):
                agout = dram.tile([NC * rows, N], f32, name=nm("agout"), tag="agout")
                nc.gpsimd.collective_compute(
                    "AllGather", Alu.bypass,
                    replica_groups=[list(range(NC))],
                    ins=[agin.opt()], outs=[agout.opt()])
                return agout

            # ---------------- branch ----------------
            lout_by_g = {}
            cancat_by_g = {}
            cnat_by_g = {}

            def branch(g):
                mask_sb = P["mask"].tile([128, NKT, N], fp8, name=nm(f"mask{g}"),
                                         tag="mask")
                nc.sync.dma_start(mask_sb[:], mask8[g][:])
                xT = None  # SBUF [128, N] for rounds 2,3
                for r in (1, 2, 3):
                    pfx = f"g{g}r{r}"
                    fka = FKg[g] if r == 1 else 1
                    wA = P["wts"].tile([128, fka, 128], f32, name=nm(f"{pfx}_wA"),
                                       tag="wts")
                    nc.sync.dma_start(wA[:], WcatA[g, r][:])
                    if r == 1:
                        get_rhs_a = lambda kt: stream_rhs(featT[g], kt)
                    else:
                        get_rhs_a = lambda kt: xT
                    frA = mm_accum_to_sb("fr", fka, get_rhs_a,
                                         lambda kt: wA[:, kt, :], 128)
                    if debug and g == 1 and r == 1:
                        d = dbg_out("frA11", [128, N])
                        nc.sync.dma_start(d[:], frA[:128, :])
                    aginA = dram.tile([64, N], f32, name=nm(f"{pfx}_aginA"),
                                      tag="agin")
                    pv_sb, f1b_dbg, f2s_dbg = attention(
                        pfx + "A", g, frA,
                        load_sm(avec[f"a1A{g}{r}"], [64, 1]),
                        load_sm(avec[f"a2A{g}{r}"], [64, 1]),
                        load_sm64(avec[f"bA{g}{r}"]),
                        mask_sb, True, False, aginA)
                    if debug and g == 1 and r == 1:
                        d = dbg_out("f1b11", [128, N])
                        nc.sync.dma_start(d[:], f1b_dbg[:128, :])
                        d = dbg_out("pvA11", [65, N])
                        nc.sync.dma_start(d[:], pv_sb[:65, :])
                        d = dbg_out("valsA11", [64, N])
                        nc.sync.dma_start(d[:], aginA[:, :])
                    agoutA = allgather(aginA, 64)

                    # GCN (independent work that fills the AG wait)
                    fkg = FKg[g] if r == 1 else 1
                    wG = P["wts"].tile([128, fkg, 64], f32, name=nm(f"{pfx}_wG"),
                                       tag="wts")
                    nc.sync.dma_start(wG[:], Wgc[g, r][:])
                    if r == 1:
                        get_rhs_g = lambda kt: stream_rhs(featT[g], kt)
                    else:
                        get_rhs_g = lambda kt: xT
                    xw = mm_accum_to_sb("xw", fkg, get_rhs_g,
                                        lambda kt: wG[:, kt, :], 64)
                    xwnat = transposes(xw, False)
                    hg = psm.tile([64, RC], f32, name=nm(f"{pfx}_hg"), tag="psm")
                    for kt in range(NKT):
                        at = P["rhs"].tile([128, N], f32, name=nm("adjt"), tag="rhs")
                        nc.sync.dma_start(at[:, :RC], adjTj[g][:, kt, :])
                        nc.tensor.matmul(hg[:], xwnat[:, kt, 0:64], at[:, :RC],
                                         start=(kt == 0), stop=(kt == NKT - 1),
                                         skip_group_check=True)
                    hgs = P["sm"].tile([64, RC], f32, name=nm(f"{pfx}_hgs"),
                                       tag="hg", bufs=2)
                    if r < 3:
                        nc.scalar.activation(hgs[:], hg[:], Act.Relu)
                    else:
                        nc.vector.tensor_copy(hgs[:], hg[:])

                    # B layer: rhs tiles reloaded from agoutA
                    wB = P["wts"].tile([128, 4, 128], f32, name=nm(f"{pfx}_wB"),
                                       tag="wts")
                    nc.sync.dma_start(wB[:], WcatB[g, r][:])
                    frB = mm_accum_to_sb("fr", 4,
                                         lambda kt: stream_rhs(agoutA, kt),
                                         lambda kt: wB[:, kt, :], 128)
                    aginB = dram.tile([72, N], f32, name=nm(f"{pfx}_aginB"),
                                      tag="agin")
                    attention(pfx + "B", g, frB,
                              load_sm(avec[f"a1B{g}{r}"], [64, 1]),
                              load_sm(avec[f"a2B{g}{r}"], [64, 1]),
                              load_sm64(avec[f"bB{g}{r}"]),
                              mask_sb, False, True, aginB)
                    # hgcn piece rides AG#2 as rows 64:72
                    nc.sync.dma_start(
                        aginB[64:72, :].rearrange("a (p c) -> (a p) c", c=RC),
                        hgs[:])
                    agoutB = allgather(aginB, 72)

                    # assemble l = [hgcn ; mean_heads(valsB)]
                    lnew = P["lout"].tile([128, N], f32, name=nm(f"{pfx}_l"),
                                          tag="lout")
                    for j in range(NC):
                        nc.sync.dma_start(
                            lnew[0:64, j * RC:(j + 1) * RC],
                            agoutB[j * 72 + 64:j * 72 + 72, :].rearrange(
                                "a (p c) -> (a p) c", c=RC))
                    # mean: sum of 8 chunks (already scaled by 1/8); all operands
                    # live on partitions 64..127 to satisfy the base-partition
                    # rule, ping-ponging between scratch and lnew[64:128]
                    scratch = P["v"].tile([128, N], f32, name=nm(f"{pfx}_ms"),
                                          tag="v")
                    loads = []
                    for j in range(NC):
                        t = P["rhs"].tile([128, N], f32, name=nm("mld"), tag="rhs")
                        nc.sync.dma_start(t[64:128, :],
                                          agoutB[j * 72:j * 72 + 64, :])
                        loads.append(t)
                        if j == 1:
                            nc.vector.scalar_tensor_tensor(
                                lnew[64:128, :], loads[0][64:128, :], 0.0,
                                loads[1][64:128, :], op0=Alu.add, op1=Alu.add)
                        elif j > 1:
                            src = lnew[64:128, :] if j % 2 == 0 else scratch[64:128, :]
                            dst = scratch[64:128, :] if j % 2 == 0 else lnew[64:128, :]
                            nc.vector.scalar_tensor_tensor(
                                dst, src, 0.0, t[64:128, :],
                                op0=Alu.add, op1=Alu.add)
                    xT = lnew
                    if debug and g == 1 and r == 1:
                        d = dbg_out("l11", [128, N])
                        nc.sync.dma_start(d[:], lnew[:, :])
                lout_by_g[g] = xT

                # cancat = relu(emb) @ Wc + bc ; recon columns
                relu_emb = P["zml"].tile([128, N], f32, name=nm(f"ccre{g}"),
                                         tag="zml")
                nc.scalar.activation(relu_emb[:], xT[:], Act.Relu)
                wc = load_sm(Wc[g], [128, 64])
                cc = mm_accum_to_sb("cc", 1, lambda kt: relu_emb,
                                    lambda kt: wc[:], 64)
                bcs = load_sm(bc[g], [64, 1])
                nc.vector.tensor_scalar(cc[0:64, :], cc[0:64, :], bcs[:], None,
                                        Alu.add)
                cnat = transposes(cc, False)
                cancat_by_g[g] = cc
                cnat_by_g[g] = cnat
                # own columns of cancat via selection matmul
                ownp = psm.tile([64, RC], f32, name=nm(f"own{g}"), tag="psm")
                for kt in range(NKT):
                    st = P["rhs"].tile([128, N], f32, name=nm("selt"), tag="rhs")
                    nc.sync.dma_start(st[:, :RC], sel[:, kt, :])
                    nc.tensor.matmul(ownp[:], cnat[:, kt, 0:64], st[:, :RC],
                                     start=(kt == 0), stop=(kt == NKT - 1),
                                     skip_group_check=True)
                own = P["sm"].tile([64, RC], f32, name=nm(f"ownsb{g}"), tag="hg",
                                   bufs=2)
                nc.vector.tensor_copy(own[:], ownp[:])
                for qt in range(NKT):
                    rq = psm.tile([128, RC], f32, name=nm("rq"), tag="psm")
                    nc.tensor.matmul(rq[:], cc[0:64, qt * 128:(qt + 1) * 128],
                                     own[:], start=True, stop=True,
                                     skip_group_check=True)
                    rqs = P["sm"].tile([128, RC], f32, name=nm("rqs"), tag="rcs",
                                       bufs=3)
                    nc.vector.tensor_copy(rqs[:], rq[:])
                    nc.sync.dma_start(recon_out[g][qt * 128:(qt + 1) * 128, :],
                                      rqs[:])

            def me_half(latnat, OHg):
                mep = psm.tile([64, M], f32, name=nm("mep"), tag="psm")
                for kt in range(NKT):
                    t = P["rhs"].tile([128, N], f32, name=nm("oht"), tag="rhs")
                    nc.sync.dma_start(t[:, :M], OHg[kt * 128:(kt + 1) * 128, :])
                    nc.tensor.matmul(mep[:], latnat[:, kt, 0:64], t[:, :M],
                                     start=(kt == 0), stop=(kt == NKT - 1),
                                     skip_group_check=True)
                mes = P["me"].tile([64, M], f32, name=nm("mes"), tag="me")
                nc.vector.tensor_copy(mes[:], mep[:])
                return mes

            # ================= emit =================
            branch(1)
            # branch-1 head piece: latent = (relu(emb1@Wd1+bd1))@Wlm + blm
            wd = load_sm(Wd1, [128, 64])
            d1p = pmed.tile([128, 1024], f32, name=nm("d1p"), tag="pmed")
            d1s = P["aux"].tile([65, N], f32, name=nm("d1s"), tag="aux")
            bd1s = load_sm(bd1, [64, 1])
            emb1 = lout_by_g[1]
            for half in (0, 1):
                for c in (0, 1):
                    sl = half * 1024 + c * 512
                    nc.tensor.matmul(d1p[:64, c * 512:(c + 1) * 512], wd[:],
                                     emb1[:, sl:sl + 512], start=True, stop=True,
                                     skip_group_check=True)
                nc.scalar.activation(d1s[0:64, half * 1024:(half + 1) * 1024],
                                     d1p[:64, :], Act.Relu, bias=bd1s[:])
            wl = load_sm(Wlm, [64, 64])
            latp = pmed.tile([128, 1024], f32, name=nm("latp"), tag="pmed")
            lats = P["aux"].tile([65, N], f32, name=nm("lats"), tag="aux")
            blms = load_sm(blm, [64, 1])
            for half in (0, 1):
                for c in (0, 1):
                    nc.tensor.matmul(
                        latp[:64, c * 512:(c + 1) * 512], wl[:],
                        d1s[0:64, half * 1024 + c * 512:half * 1024 + (c + 1) * 512],
                        start=True, stop=True, skip_group_check=True)
                nc.vector.tensor_scalar(
                    lats[0:64, half * 1024:(half + 1) * 1024], latp[:64, :],
                    blms[:], None, Alu.add)
            latnat = transposes(lats, False)
            meL = me_half(latnat, OH[1])

            branch(2)
            meR = me_half(cnat_by_g[2], OH[2])

            # final MLP head (computed redundantly on every core)
            meT = P["me"].tile([128, M], f32, name=nm("meT"), tag="meT")
            nc.vector.tensor_copy(meT[0:64, :], meL[:])
            nc.vector.tensor_copy(meT[64:128, :], meR[:])
            wf = load_sm(Wf, [128, 128])
            bfs = load_sm(bf, [128, 1])
            fp = psm.tile([128, M], f32, name=nm("fp"), tag="psm")
            nc.tensor.matmul(fp[:], wf[:], meT[:], start=True, stop=True,
                             skip_group_check=True)
            fs = P["me"].tile([128, M], f32, name=nm("fs"), tag="meT")
            nc.scalar.activation(fs[:], fp[:], Act.Relu, bias=bfs[:])
            wo = load_sm(Wo, [128, 2])
            bos = load_sm(bo, [2, 1])
            op = psm.tile([2, M], f32, name=nm("op"), tag="psm")
            nc.tensor.matmul(op[:], wo[:], fs[:], start=True, stop=True,
                             skip_group_check=True)
            osb = P["sm"].tile([2, M], f32, name=nm("osb"), tag="hg", bufs=2)
            nc.vector.tensor_scalar(osb[:], op[:], bos[:], None, Alu.add)
            nc.sync.dma_start(head_out[:], osb[:])

    nc.compile()
    return nc, dbg


# ---------------------------------------------------------------- host driver
def _pack_inputs(feat1, adj1, bias1, feat2, adj2, bias2, GID1, GID2, params):
    """Build the per-core in_maps."""
    feats = {1: np.asarray(feat1, np.float32), 2: np.asarray(feat2, np.float32)}
    adjs = {1: np.asarray(adj1, np.float32), 2: np.asarray(adj2, np.float32)}
    biases = {1: np.asarray(bias1, np.float32), 2: np.asarray(bias2, np.float32)}
    GIDs = {1: np.asarray(GID1).astype(np.int64), 2: np.asarray(GID2).astype(np.int64)}
    FP = {1: F1P, 2: F2P}

    shared = {}
    for g in (1, 2):
        shared[f"featT{g}"] = np.ascontiguousarray(
            _pad_rows(feats[g].T.astype(np.float32), FP[g]))
        m = np.where(biases[g] < -1.0, -240.0, 0.0).astype(np.float32)
        shared[f"mask8{g}"] = _t128(m).astype(ml_dtypes.float8_e4m3)
        oh = np.zeros((N, M), np.float32)
        oh[GIDs[g], np.arange(M)] = 1.0
        shared[f"OH{g}"] = oh

    def gp(g):
        return params[f"g{g}"]

    for g in (1, 2):
        p = gp(g)
        for r, (gck, gak) in enumerate(
                [("Wgc1", "gat1"), ("Wgc2", "gat2"), ("Wgc3", "gat3")], start=1):
            wgc = np.asarray(p[gck], np.float32)
            fpad = FP[g] if r == 1 else 128
            shared[f"Wgc{g}{r}"] = _t128(_pad_rows(wgc, fpad))
        shared[f"Wc{g}"] = np.asarray(p["Wc"], np.float32)
        shared[f"bc{g}"] = np.asarray(p["bc"], np.float32).reshape(64, 1)
    shared["Wd1"] = np.asarray(params["Wd1"], np.float32)
    shared["bd1"] = np.asarray(params["bd1"], np.float32).reshape(64, 1)
    shared["Wlm"] = np.asarray(params["Wlm"], np.float32)
    shared["blm"] = np.asarray(params["blm"], np.float32).reshape(64, 1)
    shared["Wf"] = np.asarray(params["Wf"], np.float32)
    shared["bf"] = np.asarray(params["bf"], np.float32).reshape(128, 1)
    shared["Wo"] = np.asarray(params["Wo"], np.float32)
    shared["bo"] = np.asarray(params["bo"], np.float32).reshape(2, 1)

    in_maps = []
    for j in range(NC):
        im = {}
        for g in (1, 2):
            im[f"featT{g}"] = shared[f"featT{g}"]
            im[f"mask8{g}"] = shared[f"mask8{g}"]
            im[f"OH{g}"] = shared[f"OH{g}"]
            im[f"adjTj{g}"] = _t128(
                np.ascontiguousarray(adjs[g].T[:, j * RC:(j + 1) * RC]))
            p = gp(g)
            for r, gak in enumerate(["gat1", "gat2", "gat3"], start=1):
                gt = p[gak]
                fpad = FP[g] if r == 1 else 128
                wa = np.concatenate([np.asarray(gt["A_W"][j], np.float32),
                                     np.asarray(gt["A_Wres"][j], np.float32)], 1)
                im[f"WcatA{g}{r}"] = _t128(_pad_rows(wa, fpad))
                wb = np.concatenate([np.asarray(gt["B_W"][j], np.float32),
                                     np.asarray(gt["B_Wres"][j], np.float32) / 8.0],
                                    1)
                im[f"WcatB{g}{r}"] = _t128(wb)
                im[f"a1A{g}{r}"] = np.asarray(gt["A_a1"][j], np.float32).reshape(64, 1)
                im[f"a2A{g}{r}"] = np.asarray(gt["A_a2"][j], np.float32).reshape(64, 1)
                im[f"bA{g}{r}"] = np.asarray(gt["A_b"][j], np.float32).reshape(64, 1)
                im[f"a1B{g}{r}"] = np.asarray(gt["B_a1"][j], np.float32).reshape(64, 1)
                im[f"a2B{g}{r}"] = np.asarray(gt["B_a2"][j], np.float32).reshape(64, 1)
                im[f"bB{g}{r}"] = (np.asarray(gt["B_b"][j], np.float32) / 8.0
                                   ).reshape(64, 1)
                im[f"Wgc{g}{r}"] = shared[f"Wgc{g}{r}"]
            im[f"Wc{g}"] = shared[f"Wc{g}"]
            im[f"bc{g}"] = shared[f"bc{g}"]
        selm = np.zeros((N, RC), np.float32)
        selm[j * RC + np.arange(RC), np.arange(RC)] = 1.0
        im["sel"] = _t128(selm)
        for k in ("Wd1", "bd1", "Wlm", "blm", "Wf", "bf", "Wo", "bo"):
            im[k] = shared[k]
        s16 = np.zeros((16, NKT * 64), np.float32)
        for kt in range(NKT):
            s16[kt, kt * 64:(kt + 1) * 64] = 1.0
        im["sel16"] = s16
        in_maps.append(im)
    return in_maps


def kernel(feat1, adj1, bias1, feat2, adj2, bias2, GID1, GID2, params,
           trace=False, debug=False):
    key = ("prog", bool(debug))
    if key not in _cache:
        _cache[key] = _build(debug=debug)
    nc, dbg = _cache[key]
    in_maps = _pack_inputs(feat1, adj1, bias1, feat2, adj2, bias2,
                           GID1, GID2, params)
    kw = {}
    if trace:
        kw = dict(trace=True, trace_cores=list(range(NC)))
    res = run_bass_kernel_spmd(nc, in_maps, core_ids=list(range(NC)), **kw)
    r0 = res.results[0]
    recon1 = np.concatenate([res.results[j]["recon1"] for j in range(NC)], axis=1)
    recon2 = np.concatenate([res.results[j]["recon2"] for j in range(NC)], axis=1)
    out = r0["headT"].T  # [M, 2]
    full = np.concatenate([recon1.reshape(-1), recon2.reshape(-1),
                           out.reshape(-1)]).astype(np.float32)
    kernel.last_results = res
    return full
